# revision 106
# baseline (speedup 1.0000x reference)
"""Trainium2 Bass kernel for banded multi-head attention (nn_MultiHeadAttention).

Full inputs in, full outputs out. Sharding: data-parallel over batch (8 cores,
one batch element each). Per core, f16 data path (PSUM always f32):

  q = (Wq*scale)^T x + bq ; k = Wk^T c + bk          (f16 SBUF via PSUM drains)
  v^T = c^T Wv^T + bv (ones-matmul bias), f16 with a ones column
  per (256-chunk i, head pair m): S^T[j,i] = k_h^T q_h  (banded njt j-tiles)
  E = exp(S^T) f16 (ACT);  E *= w  (w = mask*band/(1+|i-j|); par0 on GPSIMD,
  par1 deferred one cycle to DVE so it never delays a reciprocal)
  [numer;denom] = [V_h | 1]^T E  -> po bank(par), f32 PSUM
  one batched DVE reciprocal over both heads' denominator rows; the result is
  broadcast to 64 partitions by a DMA with a zero-stride source dim (DVE may
  read only one PSUM operand, and this keeps the broadcast off PE/ACT/DVE);
  DVE multiply (PSUM numer x SBUF rbc) -> o f16.
  final = Wo^T o + bo (f32), drained+DMA'd in 256-col quarters; the last
  quarter contracts odd-head rows straight from the o64c staging tile via a
  relaid Wo copy (wod) and adds bo with a rank-1 ones-matmul.

Emission is software-pipelined: each attention pair's PV/normalize lags its
scores/exp by THREE pair-cycles (the GPSIMD w-multiply is slow), and all
projection slices / V tiles / output-projection quarters are interleaved into
the attention stream as deadline-scheduled filler so the PE never waits on
ACT/DVE. Emission order is the Tile dependency contract: "post" fillers that
read earlier chunks' outputs must be emitted after the lagged pair's muls.
"""
import numpy as np

B, CH, T = 8, 512, 1024
H, KC, BLOCK = 8, 64, 256
P = 128
CB = CH // P       # 4 channel blocks
TTN = T // P       # 8 t-tiles
CHUNK = 256
NCH = T // CHUNK   # 4 chunks
VW = 66            # per-head V row width: 64 data + 1 ones + 1 pad

_CACHE = {}


def _chunk_jts(ch):
    jt0 = max(0, 2 * ch - 2)
    jt1 = min(TTN, 2 * ch + 4)
    return jt0, jt1


def _build_nc():
    import concourse.bass as bass
    import concourse.mybir as mybir
    import concourse.tile as tile
    from concourse import bacc

    f32 = mybir.dt.float32
    f32r = mybir.dt.float32r
    f16 = mybir.dt.float16
    AF = mybir.ActivationFunctionType

    nc = bacc.Bacc("TRN2", target_bir_lowering=False, debug=False)
    x_d = nc.dram_tensor("x", [CH, T], f16, kind="ExternalInput")
    c_d = nc.dram_tensor("c", [CH, T], f16, kind="ExternalInput")
    wq_d = nc.dram_tensor("wqt", [CH, CH], f16, kind="ExternalInput")
    wk_d = nc.dram_tensor("wkt", [CH, CH], f16, kind="ExternalInput")
    wv_d = nc.dram_tensor("wvt", [CH, CH], f16, kind="ExternalInput")
    wo_d = nc.dram_tensor("wot", [CH, CH], f16, kind="ExternalInput")
    bqko_d = nc.dram_tensor("bqko", [P, 3 * CB], f32, kind="ExternalInput")
    bv_d = nc.dram_tensor("bv", [1, CH], f16, kind="ExternalInput")
    w_d = nc.dram_tensor("w", [P, TTN, T], f16, kind="ExternalInput")
    wod_d = nc.dram_tensor("wod", [64, CB, CH], f16, kind="ExternalInput")
    bo16_d = nc.dram_tensor("bo16", [1, CH], f16, kind="ExternalInput")
    out_d = nc.dram_tensor("out", [CH, T], f32, kind="ExternalOutput")

    def cbt(dram):  # [CH, X] dram -> [P, CB, X] load view
        return dram.rearrange("(cb p) t -> p cb t", p=P)

    with tile.TileContext(nc) as tc:
        with (
            tc.tile_pool(name="const", bufs=1) as const,
            tc.tile_pool(name="work", bufs=4) as work,
            tc.tile_pool(name="epool", bufs=8) as epool,
            tc.tile_pool(name="psS", bufs=2, space="PSUM") as psS,
            tc.tile_pool(name="psO", bufs=2, space="PSUM") as psO,
        ):
            # ---------- constants & inputs ----------
            x_sb = const.tile([P, CB, T], f16)
            c_sb = const.tile([P, CB, T], f16)
            wq_sb = const.tile([P, CB, CH], f16)
            wk_sb = const.tile([P, CB, CH], f16)
            wv_sb = const.tile([P, CB, CH], f16)
            wo_sb = const.tile([P, CB, CH], f16)
            bqko_sb = const.tile([P, 3 * CB], f32)
            bv_sb = const.tile([1, CH], f16)
            w_sb = const.tile([P, TTN, T], f16)
            wod_sb = const.tile([64, CB, CH], f16)
            ones16 = const.tile([1, 256], f16)
            bo16_sb = const.tile([1, CH], f16)

            # DMA order is the compute-start order; spread across issue queues.
            # First wave split finer across 4 queues so Q-proj starts sooner.
            qs = [nc.sync, nc.gpsimd, nc.scalar]
            nc.sync.dma_start(out=x_sb[:, 0:2, 0:512], in_=cbt(x_d)[:, 0:2, 0:512])
            nc.scalar.dma_start(out=x_sb[:, 2:4, 0:512], in_=cbt(x_d)[:, 2:4, 0:512])
            nc.gpsimd.dma_start(out=wq_sb[:, :, 0:128], in_=cbt(wq_d)[:, :, 0:128])
            dmas = []
            dmas.append((c_sb[:, :, 0:512], cbt(c_d)[:, :, 0:512]))
            dmas.append((wk_sb[:, :, 0:128], cbt(wk_d)[:, :, 0:128]))
            dmas.append((bqko_sb, bqko_d[:, :]))
            dmas.append((wq_sb[:, :, 128:512], cbt(wq_d)[:, :, 128:512]))
            dmas.append((wk_sb[:, :, 128:512], cbt(wk_d)[:, :, 128:512]))
            dmas.append((wv_sb, cbt(wv_d)))
            dmas.append((bv_sb, bv_d[:, :]))
            dmas.append((c_sb[:, :, 512:T], cbt(c_d)[:, :, 512:T]))
            dmas.append((w_sb[:, 0:4, :], w_d[:, 0:4, :]))
            dmas.append((x_sb[:, :, 512:T], cbt(x_d)[:, :, 512:T]))
            dmas.append((w_sb[:, 4:TTN, :], w_d[:, 4:TTN, :]))
            dmas.append((wo_sb, cbt(wo_d)))
            dmas.append((wod_sb, wod_d[:, :, :]))
            dmas.append((bo16_sb, bo16_d[:, :]))
            for i, (dst, src) in enumerate(dmas):
                qs[i % len(qs)].dma_start(out=dst, in_=src)
            bq_sb = bqko_sb[:, 0:CB]
            bk_sb = bqko_sb[:, CB:2 * CB]
            bo_sb = bqko_sb[:, 2 * CB:3 * CB]

            q_sb = const.tile([P, CB, T], f16)
            k_sb = const.tile([P, CB, T], f16)
            v_sb = const.tile([P, TTN, H, VW], f16)
            o_sb = const.tile([P, CB, T], f16)
            nc.vector.memset(ones16, 1.0)
            nc.vector.memset(v_sb[:, :, :, 64:65], 1.0)
            # dummy exp at t=0 pulls the ACT table load off the first real
            # exp's critical path
            warm = work.tile([1, 8], f32, tag="warm", name="warm", bufs=1)
            nc.vector.memset(warm, 0.0)
            nc.scalar.activation(warm, warm, AF.Exp)
            # dummy matmuls during the initial DMA wait ramp the PE p-state
            # (HAM warm-up) so the first projections run at full clock
            wps = psS.tile([P, 256], f32, tag="s", name="wps")
            for _ in range(18):
                nc.tensor.matmul(
                    wps, ones16[0:1, 0:128], ones16[0:1, :], start=True, stop=True
                )

            # ---------- unit emitters ----------
            def proj_unit(wsb, bias_sb, src, dst, ob, t2):
                """One [128 out-ch, 512 t] slice of a Q/K projection."""
                tsl = slice(t2 * 512, (t2 + 1) * 512)
                pq = psS.tile([P, 512], f32, tag="s", name="pq")
                for cb in range(CB):
                    nc.tensor.matmul(
                        pq,
                        wsb[:, cb, ob * P:(ob + 1) * P],
                        src[:, cb, tsl],
                        start=(cb == 0),
                        stop=(cb == CB - 1),
                    )
                nc.vector.tensor_scalar_add(
                    dst[:, ob, tsl], pq, bias_sb[:, ob:ob + 1]
                )

            def v_unit(tt):
                """V^T projection for one 128-row t-tile, bias via ones-matmul."""
                pv = psS.tile([P, 512], f32, tag="s", name="pv")
                for cb in range(CB):
                    nc.tensor.matmul(
                        pv,
                        c_sb[:, cb, tt * P:(tt + 1) * P],
                        wv_sb[:, cb, :],
                        start=(cb == 0),
                        stop=False,
                    )
                nc.tensor.matmul(pv, ones16[0:1, 0:P], bv_sb, start=False, stop=True)
                nc.scalar.activation(
                    v_sb[:, tt, :, 0:64],
                    pv.rearrange("p (h d) -> p h d", h=H),
                    AF.Copy,
                )

            out_view = out_d.rearrange("(cb p) t -> p cb t", p=P)
            fin_tiles = {}

            def outproj_unit(t2, th, ob, dma_split=False):
                """One [128 out-ch, 256 t] slice of the output projection;
                DMAs its 256-col quarter when the 4th ob completes (or per-ob
                pieces for the final tail quarter)."""
                if (t2, th) not in fin_tiles:
                    fin_tiles[(t2, th)] = work.tile(
                        [P, CB, 256], f32, tag="fin", name="fin", bufs=2
                    )
                fin = fin_tiles[(t2, th)]
                csl = slice(t2 * 512 + th * 256, t2 * 512 + (th + 1) * 256)
                pf = psS.tile([P, 256], f32, tag="s", name="pf")
                for cb in range(CB):
                    nc.tensor.matmul(
                        pf,
                        wo_sb[:, cb, ob * P:(ob + 1) * P],
                        o_sb[:, cb, csl],
                        start=(cb == 0),
                        stop=(cb == CB - 1),
                    )
                nc.vector.tensor_scalar_add(fin[:, ob, :], pf, bo_sb[:, ob:ob + 1])
                if dma_split:
                    qs[ob % len(qs)].dma_start(
                        out=out_view[:, ob, csl], in_=fin[:, ob, :]
                    )
                elif ob == 3:
                    nc.sync.dma_start(out=out_view[:, :, csl], in_=fin)

            # ---------- startup: just Q-ob0/K-ob0 so attention starts ASAP;
            # every other projection slice runs as attention-cycle filler ----
            proj_unit(wq_sb, bq_sb, x_sb, q_sb, 0, 0)
            proj_unit(wk_sb, bk_sb, c_sb, k_sb, 0, 0)

            # ---------- attention pipeline ----------
            o64c_tiles = {}

            def scores_expmul(ch, m, dve0=False):
                jt0, jt1 = _chunk_jts(ch)
                njt = jt1 - jt0
                isl = slice(ch * CHUNK, (ch + 1) * CHUNK)
                ps_pair = []
                for par in (0, 1):
                    ps_p = psS.tile([P, 6, CHUNK], f32, tag="s", name="ps_p")
                    ps_pair.append(ps_p)
                for u in range(njt):
                    jt = jt0 + u
                    for par in (0, 1):
                        hp = par * 64
                        nc.tensor.matmul(
                            ps_pair[par][:, u, :],
                            k_sb[hp:hp + KC, m, jt * P:(jt + 1) * P],
                            q_sb[hp:hp + KC, m, isl],
                            start=True,
                            stop=True,
                        )
                return ps_pair

            def exp_emul0(ch, m, ps_pair, dve0=False):
                """Exp for par0 + its w-multiply (GPSIMD unless dve0)."""
                jt0, jt1 = _chunk_jts(ch)
                njt = jt1 - jt0
                isl = slice(ch * CHUNK, (ch + 1) * CHUNK)
                e_t = epool.tile([P, 6, CHUNK], f16, name="e_t")
                nc.scalar.activation(
                    e_t[:, 0:njt, :], ps_pair[0][:, 0:njt, :], AF.Exp
                )
                if dve0:
                    nc.vector.tensor_mul(
                        e_t[:, 0:njt, :], e_t[:, 0:njt, :], w_sb[:, jt0:jt1, isl]
                    )
                elif ch == 2:
                    # ch2 cycles have no Pool slack: split the multiply so
                    # GPSIMD carries 4 planes and DVE the other 2
                    nc.gpsimd.tensor_mul(
                        e_t[:, 0:4, :], e_t[:, 0:4, :], w_sb[:, jt0:jt0 + 4, isl]
                    )
                    nc.vector.tensor_mul(
                        e_t[:, 4:njt, :], e_t[:, 4:njt, :], w_sb[:, jt0 + 4:jt1, isl]
                    )
                else:
                    nc.gpsimd.tensor_mul(
                        e_t[:, 0:njt, :], e_t[:, 0:njt, :], w_sb[:, jt0:jt1, isl]
                    )
                return e_t

            def exp1(ch, m, ps_pair):
                """Exp for par1; its w-multiply is deferred one cycle (emul1)
                so it never delays a reciprocal on the DVE queue."""
                jt0, jt1 = _chunk_jts(ch)
                njt = jt1 - jt0
                e_t = epool.tile([P, 6, CHUNK], f16, name="e_t")
                nc.scalar.activation(
                    e_t[:, 0:njt, :], ps_pair[1][:, 0:njt, :], AF.Exp
                )
                return e_t

            def emul1(st):
                ch, m, e_pair = st
                jt0, jt1 = _chunk_jts(ch)
                njt = jt1 - jt0
                isl = slice(ch * CHUNK, (ch + 1) * CHUNK)
                nc.vector.tensor_mul(
                    e_pair[1][:, 0:njt, :], e_pair[1][:, 0:njt, :],
                    w_sb[:, jt0:jt1, isl],
                )

            def pv_recip(st):
                ch, m, e_pair = st
                jt0, jt1 = _chunk_jts(ch)
                njt = jt1 - jt0
                po = psO.tile([P, 2, CHUNK], f32, tag="po", name="po")
                for par in (0, 1):
                    for u in range(njt):
                        jt = jt0 + u
                        nc.tensor.matmul(
                            po[0:65, par, :],
                            v_sb[:, jt, 2 * m + par, 0:65],
                            e_pair[par][:, u, :],
                            start=(u == 0),
                            stop=(u == njt - 1),
                        )
                st.append(po)

            def recip_emit(st):
                ch, m, e_pair, po = st
                r2 = work.tile([65, 2, CHUNK], f32, tag="r2", name="r2", bufs=3)
                nc.vector.reciprocal(r2[64:65, :, :], po[64:65, :, :])
                st.append(r2)

            def pbcast(st):
                """Broadcast both reciprocal rows to 64 partitions via the
                (idle) DMA engines — DVE can read only one PSUM operand, and
                this keeps the broadcast off PE/ACT/DVE/GPSIMD entirely. The
                source re-reads partition 64 via a zero-stride free dim."""
                ch, m, e_pair, po, r2 = st
                rbc = work.tile([64, 2, CHUNK], f32, tag="rbc", name="rbc", bufs=3)
                src = r2[64:65, :, :]
                ap = list(src.ap)
                bsrc = bass.AP(src.tensor, src.offset, [ap[0], [0, 64]] + ap[1:])
                nc.sync.dma_start(out=rbc, in_=bsrc)
                st.append(rbc)

            def bcast_muls(st):
                ch, m, e_pair, po, r2, rbc = st
                isl = slice(ch * CHUNK, (ch + 1) * CHUNK)
                if m == 0:
                    o64c_tiles[ch] = work.tile(
                        [64, CB, CHUNK], f16, tag="o64c", name="o64c", bufs=2
                    )
                o64c = o64c_tiles[ch]
                nc.vector.tensor_mul(
                    o_sb[0:64, m, isl], po[0:64, 0, :], rbc[:, 0, :]
                )
                nc.vector.tensor_mul(
                    o64c[:, m, :], po[0:64, 1, :], rbc[:, 1, :]
                )
                if ch != NCH - 1 and m == H // 2 - 1:
                    # last chunk skips the gather DMA: the tail outproj reads
                    # its odd-head rows straight out of o64c via wod_sb
                    nc.sync.dma_start(out=o_sb[64:128, :, isl], in_=o64c)

            # filler units per (ch, m) pair-cycle: PE work that overlaps the
            # exp/emul of the current pair. "pre" units depend only on DMA'd
            # inputs and may be emitted between PV and bcast; "post" units
            # read earlier chunks' o_sb and must be emitted after the lagged
            # pair's muls (emission order defines Tile's dependency order).
            def F(pre=(), post=()):
                return list(pre), list(post)

            def Q(ob, t2):
                return lambda: proj_unit(wq_sb, bq_sb, x_sb, q_sb, ob, t2)

            def Kp(ob, t2):
                return lambda: proj_unit(wk_sb, bk_sb, c_sb, k_sb, ob, t2)

            def V(tt):
                return lambda: v_unit(tt)

            def OP(t2, th, ob):
                return lambda: outproj_unit(t2, th, ob)

            # placement constraints: scr(ch,m) needs Q/K ob=m for every t2
            # half the chunk's band touches; PV(ch,0) (two cycles after
            # scr(ch,0)) needs all its V j-tiles; OP units trail their chunk.
            fillers = {
                (0, 0): F(pre=[Q(1, 0), Kp(1, 0), V(0), V(1)]),
                (0, 1): F(pre=[Q(2, 0), Kp(2, 0), V(2), V(3)]),
                (0, 2): F(pre=[Q(3, 0), Kp(3, 0), Kp(0, 1)]),
                (0, 3): F(pre=[Kp(1, 1), Kp(2, 1)]),
                (1, 0): F(pre=[Kp(3, 1), V(4)]),
                (1, 1): F(pre=[Q(0, 1), V(5)]),
                (1, 2): F(pre=[Q(1, 1), V(6)],
                          post=[OP(0, 0, 0)]),
                (1, 3): F(pre=[Q(2, 1), V(7)],
                          post=[OP(0, 0, 1)]),
                (2, 0): F(pre=[Q(3, 1)], post=[OP(0, 0, 2)]),
                (2, 1): F(post=[OP(0, 0, 3)]),
                (2, 2): F(post=[OP(0, 1, 0)]),
                (2, 3): F(post=[OP(0, 1, 1)]),
                (3, 0): F(post=[OP(0, 1, 2)]),
                (3, 1): F(post=[OP(0, 1, 3)]),
                (3, 2): F(post=[OP(1, 0, 0), OP(1, 0, 1)]),
                (3, 3): F(post=[OP(1, 0, 2), OP(1, 0, 3)]),
            }

            # PV/normalize lag scores by TWO pair-cycles so the slow GPSIMD
            # e-mul (par0) is off the critical path. Cycle order: PV+recip of
            # the lagged pair, a filler to cover the reciprocal latency, then
            # its bcast+muls, then the current pair's scores/exp/emul.
            from collections import deque
            pending = deque()
            for ch in range(NCH):
                for m in range(H // 2):
                    done = pending.popleft() if len(pending) == 3 else None
                    pre, post = fillers[(ch, m)]
                    if done is not None:
                        pv_recip(done)
                        recip_emit(done)
                    if pre:
                        pre.pop(0)()
                    ps_pair = scores_expmul(ch, m)
                    if done is not None:
                        pbcast(done)
                    e0 = exp_emul0(
                        ch, m, ps_pair,
                        dve0=(ch, m) >= (3, 2),
                    )
                    if pending:
                        emul1(pending[-1])
                    if done is not None:
                        bcast_muls(done)
                    e1 = exp1(ch, m, ps_pair)
                    for f in pre + post:
                        f()
                    pending.append([ch, m, [e0, e1]])

            # ---------- drain + last output-projection quarter ----------
            # The tail outproj accumulates channel blocks 0-2 (heads 0-5, all
            # landed) first, holding cb3 until the final pair's muls finish.
            st_a = pending.popleft()
            st_b = pending.popleft()
            st_c = pending.popleft()
            fin = work.tile([P, CB, 256], f32, tag="fin", name="fin", bufs=2)
            csl = slice(768, 1024)
            o64c3 = o64c_tiles[NCH - 1]
            # two tag-s slots give each ob its own PSUM bank, so all four obs
            # accumulate incrementally as each remaining pair lands; odd-head
            # rows contract straight from o64c (no gather DMA for chunk 3)
            pfab = psS.tile([P, 2, 512], f32, tag="s", name="pfab")
            pfcd = psS.tile([P, 2, 512], f32, tag="s", name="pfcd")

            def tail_cb(cb, stop):
                for ob in range(CB):
                    pf = (pfab, pfcd)[ob // 2][0:P, ob % 2, 0:256]
                    nc.tensor.matmul(
                        pf,
                        wo_sb[0:64, cb, ob * P:(ob + 1) * P],
                        o_sb[0:64, cb, csl],
                        start=(cb == 0),
                        stop=False,
                    )
                    if cb == 0:
                        # bias via rank-1 ones-matmul so the drain is a plain
                        # (batchable) copy
                        nc.tensor.matmul(
                            pf,
                            bo16_sb[0:1, ob * P:(ob + 1) * P],
                            ones16[0:1, :],
                            start=False,
                            stop=False,
                        )
                    nc.tensor.matmul(
                        pf,
                        wod_sb[:, cb, ob * P:(ob + 1) * P],
                        o64c3[:, cb, :],
                        start=False,
                        stop=stop,
                    )

            emul1(st_c)
            pv_recip(st_a)
            recip_emit(st_a)
            pv_recip(st_b)
            recip_emit(st_b)
            tail_cb(0, False)
            pbcast(st_a)
            bcast_muls(st_a)
            pv_recip(st_c)
            recip_emit(st_c)
            tail_cb(1, False)
            pbcast(st_b)
            bcast_muls(st_b)
            tail_cb(2, False)
            pbcast(st_c)
            bcast_muls(st_c)
            tail_cb(3, True)
            nc.vector.tensor_copy(fin[:, 0, :], pfab[0:P, 0, 0:256])
            nc.scalar.activation(fin[:, 2, :], pfcd[0:P, 0, 0:256], AF.Copy)
            nc.sync.dma_start(out=out_view[:, 0, csl], in_=fin[:, 0, :])
            nc.gpsimd.dma_start(out=out_view[:, 2, csl], in_=fin[:, 2, :])
            nc.vector.tensor_copy(fin[:, 1, :], pfab[0:P, 1, 0:256])
            nc.scalar.activation(fin[:, 3, :], pfcd[0:P, 1, 0:256], AF.Copy)
            nc.sync.dma_start(out=out_view[:, 1, csl], in_=fin[:, 1, :])
            nc.scalar.dma_start(out=out_view[:, 3, csl], in_=fin[:, 3, :])

    nc.compile()
    return nc


def _host_prep(attn_mask, Wq, bq, Wk, bk, Wv, bv, Wo, bo):
    """Precompute per-core shared inputs (f16 weight layouts + combined
    band/bias/mask weight matrix)."""
    scale = 1.0 / np.sqrt(KC)
    wqt = np.ascontiguousarray((np.asarray(Wq) * scale).T.astype(np.float16))
    wkt = np.ascontiguousarray(np.asarray(Wk).T.astype(np.float16))
    wvt = np.ascontiguousarray(np.asarray(Wv).T.astype(np.float16))
    wot = np.ascontiguousarray(np.asarray(Wo).T.astype(np.float16))
    # odd-head rows of Wo^T relaid onto partitions 0:64, for the tail outproj
    wod = np.ascontiguousarray(
        wot.reshape(CB, P, CH)[:, 64:128, :].transpose(1, 0, 2)
    )
    bqko = np.concatenate(
        [
            (np.asarray(bq) * scale).astype(np.float32).reshape(CB, P).T,
            np.asarray(bk).astype(np.float32).reshape(CB, P).T,
            np.asarray(bo).astype(np.float32).reshape(CB, P).T,
        ],
        axis=1,
    )
    bqko = np.ascontiguousarray(bqko)
    bv_r = np.ascontiguousarray(np.asarray(bv).astype(np.float16).reshape(1, CH))

    r = np.arange(T)
    diff = np.abs(r[None, :] - r[:, None])            # |i - j|
    w_mat = 1.0 / (1.0 + diff.astype(np.float64))      # exp(-log1p|i-j|)
    band = diff <= BLOCK
    mask = np.asarray(attn_mask).reshape(T, T) != 0    # [i, j]
    w_eff = np.where(band & mask, w_mat, 0.0)          # [i, j]
    w_T = w_eff.T                                      # [j, i]
    w_planes = np.ascontiguousarray(
        w_T.reshape(TTN, P, T).transpose(1, 0, 2).astype(np.float16)
    )
    bo16 = np.ascontiguousarray(np.asarray(bo).astype(np.float16).reshape(1, CH))
    return dict(
        wqt=wqt, wkt=wkt, wvt=wvt, wot=wot, wod=wod,
        bqko=bqko, bv=bv_r, bo16=bo16, w=w_planes,
    )


def kernel(x, c, attn_mask, Wq, bq, Wk, bk, Wv, bv, Wo, bo, _trace=False):
    from concourse.bass_utils import run_bass_kernel_spmd

    if "nc" not in _CACHE:
        _CACHE["nc"] = _build_nc()
    nc = _CACHE["nc"]

    shared = _host_prep(attn_mask, Wq, bq, Wk, bk, Wv, bv, Wo, bo)
    x = np.asarray(x, dtype=np.float16)
    c = np.asarray(c, dtype=np.float16)
    in_maps = [
        dict(shared, x=np.ascontiguousarray(x[b]), c=np.ascontiguousarray(c[b]))
        for b in range(B)
    ]
    kwargs = {}
    if _trace:
        kwargs = dict(trace=True)
    res = run_bass_kernel_spmd(nc, in_maps, core_ids=list(range(B)), **kwargs)
    out = np.stack([res.results[b]["out"] for b in range(B)], axis=0)
    if _trace:
        _CACHE["last_results"] = res
    return out


# revision 107
# speedup vs baseline: 1.0042x; 1.0042x over previous
"""Trainium2 Bass kernel for banded multi-head attention (nn_MultiHeadAttention).

Full inputs in, full outputs out. Sharding: data-parallel over batch (8 cores,
one batch element each). Per core, f16 data path (PSUM always f32):

  q = (Wq*scale)^T x + bq ; k = Wk^T c + bk          (f16 SBUF via PSUM drains)
  v^T = c^T Wv^T + bv (ones-matmul bias), f16 with a ones column
  per (256-chunk i, head pair m): S^T[j,i] = k_h^T q_h  (banded njt j-tiles)
  E = exp(S^T) f16 (ACT);  E *= w  (w = mask*band/(1+|i-j|); par0 on GPSIMD,
  par1 deferred one cycle to DVE so it never delays a reciprocal)
  [numer;denom] = [V_h | 1]^T E  -> po bank(par), f32 PSUM
  one batched DVE reciprocal over both heads' denominator rows; the result is
  broadcast to 64 partitions by a DMA with a zero-stride source dim (DVE may
  read only one PSUM operand, and this keeps the broadcast off PE/ACT/DVE);
  DVE multiply (PSUM numer x SBUF rbc) -> o f16.
  final = Wo^T o + bo (f32), drained+DMA'd in 256-col quarters; the last
  quarter contracts odd-head rows straight from the o64c staging tile via a
  relaid Wo copy (wod) and adds bo with a rank-1 ones-matmul.

Emission is software-pipelined: each attention pair's PV/normalize lags its
scores/exp by THREE pair-cycles (the GPSIMD w-multiply is slow), and all
projection slices / V tiles / output-projection quarters are interleaved into
the attention stream as deadline-scheduled filler so the PE never waits on
ACT/DVE. Emission order is the Tile dependency contract: "post" fillers that
read earlier chunks' outputs must be emitted after the lagged pair's muls.
"""
import numpy as np

B, CH, T = 8, 512, 1024
H, KC, BLOCK = 8, 64, 256
P = 128
CB = CH // P       # 4 channel blocks
TTN = T // P       # 8 t-tiles
CHUNK = 256
NCH = T // CHUNK   # 4 chunks
VW = 66            # per-head V row width: 64 data + 1 ones + 1 pad

_CACHE = {}


def _chunk_jts(ch):
    jt0 = max(0, 2 * ch - 2)
    jt1 = min(TTN, 2 * ch + 4)
    return jt0, jt1


def _build_nc():
    import concourse.bass as bass
    import concourse.mybir as mybir
    import concourse.tile as tile
    from concourse import bacc

    f32 = mybir.dt.float32
    f32r = mybir.dt.float32r
    f16 = mybir.dt.float16
    AF = mybir.ActivationFunctionType

    nc = bacc.Bacc("TRN2", target_bir_lowering=False, debug=False)
    x_d = nc.dram_tensor("x", [CH, T], f16, kind="ExternalInput")
    c_d = nc.dram_tensor("c", [CH, T], f16, kind="ExternalInput")
    wq_d = nc.dram_tensor("wqt", [CH, CH], f16, kind="ExternalInput")
    wk_d = nc.dram_tensor("wkt", [CH, CH], f16, kind="ExternalInput")
    wv_d = nc.dram_tensor("wvt", [CH, CH], f16, kind="ExternalInput")
    wo_d = nc.dram_tensor("wot", [CH, CH], f16, kind="ExternalInput")
    bqko_d = nc.dram_tensor("bqko", [P, 3 * CB], f32, kind="ExternalInput")
    bv_d = nc.dram_tensor("bv", [1, CH], f16, kind="ExternalInput")
    w_d = nc.dram_tensor("w", [P, TTN, T], f16, kind="ExternalInput")
    wod_d = nc.dram_tensor("wod", [64, CB, CH], f16, kind="ExternalInput")
    bo16_d = nc.dram_tensor("bo16", [1, CH], f16, kind="ExternalInput")
    out_d = nc.dram_tensor("out", [CH, T], f32, kind="ExternalOutput")

    def cbt(dram):  # [CH, X] dram -> [P, CB, X] load view
        return dram.rearrange("(cb p) t -> p cb t", p=P)

    with tile.TileContext(nc) as tc:
        with (
            tc.tile_pool(name="const", bufs=1) as const,
            tc.tile_pool(name="work", bufs=4) as work,
            tc.tile_pool(name="epool", bufs=9) as epool,
            tc.tile_pool(name="psS", bufs=2, space="PSUM") as psS,
            tc.tile_pool(name="psO", bufs=2, space="PSUM") as psO,
        ):
            # ---------- constants & inputs ----------
            x_sb = const.tile([P, CB, T], f16)
            c_sb = const.tile([P, CB, T], f16)
            wq_sb = const.tile([P, CB, CH], f16)
            wk_sb = const.tile([P, CB, CH], f16)
            wv_sb = const.tile([P, CB, CH], f16)
            wo_sb = const.tile([P, CB, CH], f16)
            bqko_sb = const.tile([P, 3 * CB], f32)
            bv_sb = const.tile([1, CH], f16)
            w_sb = const.tile([P, TTN, T], f16)
            wod_sb = const.tile([64, CB, CH], f16)
            ones16 = const.tile([1, 256], f16)
            bo16_sb = const.tile([1, CH], f16)

            # DMA order is the compute-start order; spread across issue queues.
            # First wave split finer across 4 queues so Q-proj starts sooner.
            qs = [nc.sync, nc.gpsimd, nc.scalar]
            nc.sync.dma_start(out=x_sb[:, 0:2, 0:512], in_=cbt(x_d)[:, 0:2, 0:512])
            nc.scalar.dma_start(out=x_sb[:, 2:4, 0:512], in_=cbt(x_d)[:, 2:4, 0:512])
            nc.gpsimd.dma_start(out=wq_sb[:, :, 0:128], in_=cbt(wq_d)[:, :, 0:128])
            dmas = []
            dmas.append((c_sb[:, :, 0:512], cbt(c_d)[:, :, 0:512]))
            dmas.append((wk_sb[:, :, 0:128], cbt(wk_d)[:, :, 0:128]))
            dmas.append((bqko_sb, bqko_d[:, :]))
            dmas.append((wq_sb[:, :, 128:512], cbt(wq_d)[:, :, 128:512]))
            dmas.append((wk_sb[:, :, 128:512], cbt(wk_d)[:, :, 128:512]))
            dmas.append((wv_sb, cbt(wv_d)))
            dmas.append((bv_sb, bv_d[:, :]))
            dmas.append((c_sb[:, :, 512:T], cbt(c_d)[:, :, 512:T]))
            dmas.append((w_sb[:, 0:4, :], w_d[:, 0:4, :]))
            dmas.append((x_sb[:, :, 512:T], cbt(x_d)[:, :, 512:T]))
            dmas.append((w_sb[:, 4:TTN, :], w_d[:, 4:TTN, :]))
            dmas.append((wo_sb, cbt(wo_d)))
            dmas.append((wod_sb, wod_d[:, :, :]))
            dmas.append((bo16_sb, bo16_d[:, :]))
            for i, (dst, src) in enumerate(dmas):
                qs[i % len(qs)].dma_start(out=dst, in_=src)
            bq_sb = bqko_sb[:, 0:CB]
            bk_sb = bqko_sb[:, CB:2 * CB]
            bo_sb = bqko_sb[:, 2 * CB:3 * CB]

            q_sb = const.tile([P, CB, T], f16)
            k_sb = const.tile([P, CB, T], f16)
            v_sb = const.tile([P, TTN, H, VW], f16)
            o_sb = const.tile([P, CB, T], f16)
            nc.vector.memset(ones16, 1.0)
            nc.vector.memset(v_sb[:, :, :, 64:65], 1.0)
            # dummy exp at t=0 pulls the ACT table load off the first real
            # exp's critical path
            warm = work.tile([1, 8], f32, tag="warm", name="warm", bufs=1)
            nc.vector.memset(warm, 0.0)
            nc.scalar.activation(warm, warm, AF.Exp)
            # dummy matmuls during the initial DMA wait ramp the PE p-state
            # (HAM warm-up) so the first projections run at full clock
            wps = psS.tile([P, 256], f32, tag="s", name="wps")
            for _ in range(18):
                nc.tensor.matmul(
                    wps, ones16[0:1, 0:128], ones16[0:1, :], start=True, stop=True
                )

            # ---------- unit emitters ----------
            def proj_unit(wsb, bias_sb, src, dst, ob, t2):
                """One [128 out-ch, 512 t] slice of a Q/K projection."""
                tsl = slice(t2 * 512, (t2 + 1) * 512)
                pq = psS.tile([P, 512], f32, tag="s", name="pq")
                for cb in range(CB):
                    nc.tensor.matmul(
                        pq,
                        wsb[:, cb, ob * P:(ob + 1) * P],
                        src[:, cb, tsl],
                        start=(cb == 0),
                        stop=(cb == CB - 1),
                    )
                nc.vector.tensor_scalar_add(
                    dst[:, ob, tsl], pq, bias_sb[:, ob:ob + 1]
                )

            def v_unit(tt):
                """V^T projection for one 128-row t-tile, bias via ones-matmul."""
                pv = psS.tile([P, 512], f32, tag="s", name="pv")
                for cb in range(CB):
                    nc.tensor.matmul(
                        pv,
                        c_sb[:, cb, tt * P:(tt + 1) * P],
                        wv_sb[:, cb, :],
                        start=(cb == 0),
                        stop=False,
                    )
                nc.tensor.matmul(pv, ones16[0:1, 0:P], bv_sb, start=False, stop=True)
                nc.scalar.activation(
                    v_sb[:, tt, :, 0:64],
                    pv.rearrange("p (h d) -> p h d", h=H),
                    AF.Copy,
                )

            out_view = out_d.rearrange("(cb p) t -> p cb t", p=P)
            fin_tiles = {}

            def outproj_unit(t2, th, ob, dma_split=False):
                """One [128 out-ch, 256 t] slice of the output projection;
                DMAs its 256-col quarter when the 4th ob completes (or per-ob
                pieces for the final tail quarter)."""
                if (t2, th) not in fin_tiles:
                    fin_tiles[(t2, th)] = work.tile(
                        [P, CB, 256], f32, tag="fin", name="fin", bufs=3
                    )
                fin = fin_tiles[(t2, th)]
                csl = slice(t2 * 512 + th * 256, t2 * 512 + (th + 1) * 256)
                pf = psS.tile([P, 256], f32, tag="s", name="pf")
                for cb in range(CB):
                    nc.tensor.matmul(
                        pf,
                        wo_sb[:, cb, ob * P:(ob + 1) * P],
                        o_sb[:, cb, csl],
                        start=(cb == 0),
                        stop=(cb == CB - 1),
                    )
                nc.vector.tensor_scalar_add(fin[:, ob, :], pf, bo_sb[:, ob:ob + 1])
                if dma_split:
                    qs[ob % len(qs)].dma_start(
                        out=out_view[:, ob, csl], in_=fin[:, ob, :]
                    )
                elif ob == 3:
                    nc.sync.dma_start(out=out_view[:, :, csl], in_=fin)

            # ---------- startup: just Q-ob0/K-ob0 so attention starts ASAP;
            # every other projection slice runs as attention-cycle filler ----
            proj_unit(wq_sb, bq_sb, x_sb, q_sb, 0, 0)
            proj_unit(wk_sb, bk_sb, c_sb, k_sb, 0, 0)

            # ---------- attention pipeline ----------
            o64c_tiles = {}

            def scores_expmul(ch, m, dve0=False):
                jt0, jt1 = _chunk_jts(ch)
                njt = jt1 - jt0
                isl = slice(ch * CHUNK, (ch + 1) * CHUNK)
                ps_pair = []
                for par in (0, 1):
                    ps_p = psS.tile([P, 6, CHUNK], f32, tag="s", name="ps_p")
                    ps_pair.append(ps_p)
                for u in range(njt):
                    jt = jt0 + u
                    for par in (0, 1):
                        hp = par * 64
                        nc.tensor.matmul(
                            ps_pair[par][:, u, :],
                            k_sb[hp:hp + KC, m, jt * P:(jt + 1) * P],
                            q_sb[hp:hp + KC, m, isl],
                            start=True,
                            stop=True,
                        )
                return ps_pair

            def exp_emul0(ch, m, ps_pair, dve0=False):
                """Exp for par0 + its w-multiply (GPSIMD unless dve0)."""
                jt0, jt1 = _chunk_jts(ch)
                njt = jt1 - jt0
                isl = slice(ch * CHUNK, (ch + 1) * CHUNK)
                e_t = epool.tile([P, 6, CHUNK], f16, name="e_t")
                nc.scalar.activation(
                    e_t[:, 0:njt, :], ps_pair[0][:, 0:njt, :], AF.Exp
                )
                if dve0:
                    nc.vector.tensor_mul(
                        e_t[:, 0:njt, :], e_t[:, 0:njt, :], w_sb[:, jt0:jt1, isl]
                    )
                elif ch == 2:
                    # ch2 cycles have no Pool slack: split the multiply so
                    # GPSIMD carries 4 planes and DVE the other 2
                    nc.gpsimd.tensor_mul(
                        e_t[:, 0:4, :], e_t[:, 0:4, :], w_sb[:, jt0:jt0 + 4, isl]
                    )
                    nc.vector.tensor_mul(
                        e_t[:, 4:njt, :], e_t[:, 4:njt, :], w_sb[:, jt0 + 4:jt1, isl]
                    )
                else:
                    nc.gpsimd.tensor_mul(
                        e_t[:, 0:njt, :], e_t[:, 0:njt, :], w_sb[:, jt0:jt1, isl]
                    )
                return e_t

            def exp1(ch, m, ps_pair):
                """Exp for par1; its w-multiply is deferred one cycle (emul1)
                so it never delays a reciprocal on the DVE queue."""
                jt0, jt1 = _chunk_jts(ch)
                njt = jt1 - jt0
                e_t = epool.tile([P, 6, CHUNK], f16, name="e_t")
                nc.scalar.activation(
                    e_t[:, 0:njt, :], ps_pair[1][:, 0:njt, :], AF.Exp
                )
                return e_t

            def emul1(st):
                ch, m, e_pair = st
                jt0, jt1 = _chunk_jts(ch)
                njt = jt1 - jt0
                isl = slice(ch * CHUNK, (ch + 1) * CHUNK)
                nc.vector.tensor_mul(
                    e_pair[1][:, 0:njt, :], e_pair[1][:, 0:njt, :],
                    w_sb[:, jt0:jt1, isl],
                )

            def pv_recip(st):
                ch, m, e_pair = st
                jt0, jt1 = _chunk_jts(ch)
                njt = jt1 - jt0
                po = psO.tile([P, 2, CHUNK], f32, tag="po", name="po")
                for par in (0, 1):
                    for u in range(njt):
                        jt = jt0 + u
                        nc.tensor.matmul(
                            po[0:65, par, :],
                            v_sb[:, jt, 2 * m + par, 0:65],
                            e_pair[par][:, u, :],
                            start=(u == 0),
                            stop=(u == njt - 1),
                        )
                st.append(po)

            def recip_emit(st):
                ch, m, e_pair, po = st
                r2 = work.tile([65, 2, CHUNK], f32, tag="r2", name="r2", bufs=3)
                nc.vector.reciprocal(r2[64:65, :, :], po[64:65, :, :])
                st.append(r2)

            def pbcast(st):
                """Broadcast both reciprocal rows to 64 partitions via the
                (idle) DMA engines — DVE can read only one PSUM operand, and
                this keeps the broadcast off PE/ACT/DVE/GPSIMD entirely. The
                source re-reads partition 64 via a zero-stride free dim."""
                ch, m, e_pair, po, r2 = st
                rbc = work.tile([64, 2, CHUNK], f32, tag="rbc", name="rbc", bufs=3)
                src = r2[64:65, :, :]
                ap = list(src.ap)
                bsrc = bass.AP(src.tensor, src.offset, [ap[0], [0, 64]] + ap[1:])
                nc.sync.dma_start(out=rbc, in_=bsrc)
                st.append(rbc)

            def bcast_muls(st):
                ch, m, e_pair, po, r2, rbc = st
                isl = slice(ch * CHUNK, (ch + 1) * CHUNK)
                if m == 0:
                    o64c_tiles[ch] = work.tile(
                        [64, CB, CHUNK], f16, tag="o64c", name="o64c", bufs=3
                    )
                o64c = o64c_tiles[ch]
                nc.vector.tensor_mul(
                    o_sb[0:64, m, isl], po[0:64, 0, :], rbc[:, 0, :]
                )
                nc.vector.tensor_mul(
                    o64c[:, m, :], po[0:64, 1, :], rbc[:, 1, :]
                )
                if ch != NCH - 1 and m == H // 2 - 1:
                    # last chunk skips the gather DMA: the tail outproj reads
                    # its odd-head rows straight out of o64c via wod_sb
                    nc.sync.dma_start(out=o_sb[64:128, :, isl], in_=o64c)

            # filler units per (ch, m) pair-cycle: PE work that overlaps the
            # exp/emul of the current pair. "pre" units depend only on DMA'd
            # inputs and may be emitted between PV and bcast; "post" units
            # read earlier chunks' o_sb and must be emitted after the lagged
            # pair's muls (emission order defines Tile's dependency order).
            def F(pre=(), post=()):
                return list(pre), list(post)

            def Q(ob, t2):
                return lambda: proj_unit(wq_sb, bq_sb, x_sb, q_sb, ob, t2)

            def Kp(ob, t2):
                return lambda: proj_unit(wk_sb, bk_sb, c_sb, k_sb, ob, t2)

            def V(tt):
                return lambda: v_unit(tt)

            def OP(t2, th, ob):
                return lambda: outproj_unit(t2, th, ob)

            # placement constraints: scr(ch,m) needs Q/K ob=m for every t2
            # half the chunk's band touches; PV(ch,0) (two cycles after
            # scr(ch,0)) needs all its V j-tiles; OP units trail their chunk.
            fillers = {
                (0, 0): F(pre=[Q(1, 0), Kp(1, 0), V(0), V(1)]),
                (0, 1): F(pre=[Q(2, 0), Kp(2, 0), V(2), V(3)]),
                (0, 2): F(pre=[Q(3, 0), Kp(3, 0), Kp(0, 1)]),
                (0, 3): F(pre=[Kp(1, 1), Kp(2, 1)]),
                (1, 0): F(pre=[Kp(3, 1), V(4)]),
                (1, 1): F(pre=[Q(0, 1), V(5)]),
                (1, 2): F(pre=[Q(1, 1), V(6)],
                          post=[OP(0, 0, 0)]),
                (1, 3): F(pre=[Q(2, 1), V(7)],
                          post=[OP(0, 0, 1)]),
                (2, 0): F(pre=[Q(3, 1)], post=[OP(0, 0, 2)]),
                (2, 1): F(post=[OP(0, 0, 3)]),
                (2, 2): F(post=[OP(0, 1, 0)]),
                (2, 3): F(post=[OP(0, 1, 1)]),
                (3, 0): F(post=[OP(0, 1, 2)]),
                (3, 1): F(post=[OP(0, 1, 3)]),
                (3, 2): F(post=[OP(1, 0, 0), OP(1, 0, 1)]),
                (3, 3): F(post=[OP(1, 0, 2), OP(1, 0, 3)]),
            }

            # PV/normalize lag scores by TWO pair-cycles so the slow GPSIMD
            # e-mul (par0) is off the critical path. Cycle order: PV+recip of
            # the lagged pair, a filler to cover the reciprocal latency, then
            # its bcast+muls, then the current pair's scores/exp/emul.
            from collections import deque
            pending = deque()
            for ch in range(NCH):
                for m in range(H // 2):
                    done = pending.popleft() if len(pending) == 3 else None
                    pre, post = fillers[(ch, m)]
                    if done is not None:
                        pv_recip(done)
                        recip_emit(done)
                    if pre:
                        pre.pop(0)()
                    ps_pair = scores_expmul(ch, m)
                    if done is not None:
                        pbcast(done)
                    e0 = exp_emul0(
                        ch, m, ps_pair,
                        dve0=(ch, m) >= (3, 2),
                    )
                    if pending:
                        emul1(pending[-1])
                    if done is not None:
                        bcast_muls(done)
                    e1 = exp1(ch, m, ps_pair)
                    for f in pre + post:
                        f()
                    pending.append([ch, m, [e0, e1]])

            # ---------- drain + last output-projection quarter ----------
            # The tail outproj accumulates channel blocks 0-2 (heads 0-5, all
            # landed) first, holding cb3 until the final pair's muls finish.
            st_a = pending.popleft()
            st_b = pending.popleft()
            st_c = pending.popleft()
            fin = work.tile([P, CB, 256], f32, tag="fin", name="fin", bufs=3)
            csl = slice(768, 1024)
            o64c3 = o64c_tiles[NCH - 1]
            # two tag-s slots give each ob its own PSUM bank, so all four obs
            # accumulate incrementally as each remaining pair lands; odd-head
            # rows contract straight from o64c (no gather DMA for chunk 3)
            pfab = psS.tile([P, 2, 512], f32, tag="s", name="pfab")
            pfcd = psS.tile([P, 2, 512], f32, tag="s", name="pfcd")

            def tail_cb(cb, stop):
                for ob in range(CB):
                    pf = (pfab, pfcd)[ob // 2][0:P, ob % 2, 0:256]
                    nc.tensor.matmul(
                        pf,
                        wo_sb[0:64, cb, ob * P:(ob + 1) * P],
                        o_sb[0:64, cb, csl],
                        start=(cb == 0),
                        stop=False,
                    )
                    if cb == 0:
                        # bias via rank-1 ones-matmul so the drain is a plain
                        # (batchable) copy
                        nc.tensor.matmul(
                            pf,
                            bo16_sb[0:1, ob * P:(ob + 1) * P],
                            ones16[0:1, :],
                            start=False,
                            stop=False,
                        )
                    nc.tensor.matmul(
                        pf,
                        wod_sb[:, cb, ob * P:(ob + 1) * P],
                        o64c3[:, cb, :],
                        start=False,
                        stop=stop,
                    )

            emul1(st_c)
            pv_recip(st_a)
            recip_emit(st_a)
            pv_recip(st_b)
            recip_emit(st_b)
            tail_cb(0, False)
            pbcast(st_a)
            bcast_muls(st_a)
            pv_recip(st_c)
            recip_emit(st_c)
            tail_cb(1, False)
            pbcast(st_b)
            bcast_muls(st_b)
            tail_cb(2, False)
            pbcast(st_c)
            bcast_muls(st_c)
            tail_cb(3, True)
            nc.vector.tensor_copy(fin[:, 0, :], pfab[0:P, 0, 0:256])
            nc.scalar.activation(fin[:, 2, :], pfcd[0:P, 0, 0:256], AF.Copy)
            nc.sync.dma_start(out=out_view[:, 0, csl], in_=fin[:, 0, :])
            nc.gpsimd.dma_start(out=out_view[:, 2, csl], in_=fin[:, 2, :])
            nc.vector.tensor_copy(fin[:, 1, :], pfab[0:P, 1, 0:256])
            nc.scalar.activation(fin[:, 3, :], pfcd[0:P, 1, 0:256], AF.Copy)
            nc.sync.dma_start(out=out_view[:, 1, csl], in_=fin[:, 1, :])
            nc.scalar.dma_start(out=out_view[:, 3, csl], in_=fin[:, 3, :])

    nc.compile()
    return nc


def _host_prep(attn_mask, Wq, bq, Wk, bk, Wv, bv, Wo, bo):
    """Precompute per-core shared inputs (f16 weight layouts + combined
    band/bias/mask weight matrix)."""
    scale = 1.0 / np.sqrt(KC)
    wqt = np.ascontiguousarray((np.asarray(Wq) * scale).T.astype(np.float16))
    wkt = np.ascontiguousarray(np.asarray(Wk).T.astype(np.float16))
    wvt = np.ascontiguousarray(np.asarray(Wv).T.astype(np.float16))
    wot = np.ascontiguousarray(np.asarray(Wo).T.astype(np.float16))
    # odd-head rows of Wo^T relaid onto partitions 0:64, for the tail outproj
    wod = np.ascontiguousarray(
        wot.reshape(CB, P, CH)[:, 64:128, :].transpose(1, 0, 2)
    )
    bqko = np.concatenate(
        [
            (np.asarray(bq) * scale).astype(np.float32).reshape(CB, P).T,
            np.asarray(bk).astype(np.float32).reshape(CB, P).T,
            np.asarray(bo).astype(np.float32).reshape(CB, P).T,
        ],
        axis=1,
    )
    bqko = np.ascontiguousarray(bqko)
    bv_r = np.ascontiguousarray(np.asarray(bv).astype(np.float16).reshape(1, CH))

    r = np.arange(T)
    diff = np.abs(r[None, :] - r[:, None])            # |i - j|
    w_mat = 1.0 / (1.0 + diff.astype(np.float64))      # exp(-log1p|i-j|)
    band = diff <= BLOCK
    mask = np.asarray(attn_mask).reshape(T, T) != 0    # [i, j]
    w_eff = np.where(band & mask, w_mat, 0.0)          # [i, j]
    w_T = w_eff.T                                      # [j, i]
    w_planes = np.ascontiguousarray(
        w_T.reshape(TTN, P, T).transpose(1, 0, 2).astype(np.float16)
    )
    bo16 = np.ascontiguousarray(np.asarray(bo).astype(np.float16).reshape(1, CH))
    return dict(
        wqt=wqt, wkt=wkt, wvt=wvt, wot=wot, wod=wod,
        bqko=bqko, bv=bv_r, bo16=bo16, w=w_planes,
    )


def kernel(x, c, attn_mask, Wq, bq, Wk, bk, Wv, bv, Wo, bo, _trace=False):
    from concourse.bass_utils import run_bass_kernel_spmd

    if "nc" not in _CACHE:
        _CACHE["nc"] = _build_nc()
    nc = _CACHE["nc"]

    shared = _host_prep(attn_mask, Wq, bq, Wk, bk, Wv, bv, Wo, bo)
    x = np.asarray(x, dtype=np.float16)
    c = np.asarray(c, dtype=np.float16)
    in_maps = [
        dict(shared, x=np.ascontiguousarray(x[b]), c=np.ascontiguousarray(c[b]))
        for b in range(B)
    ]
    kwargs = {}
    if _trace:
        kwargs = dict(trace=True)
    res = run_bass_kernel_spmd(nc, in_maps, core_ids=list(range(B)), **kwargs)
    out = np.stack([res.results[b]["out"] for b in range(B)], axis=0)
    if _trace:
        _CACHE["last_results"] = res
    return out
